# revision 18
# baseline (speedup 1.0000x reference)
"""CRF forward (alpha) recursion on 8 Trainium2 NeuronCores.

Strategy
--------
Data-parallel over batch: each core gets 32 of 256 batch rows.

Per core, the T=512 recurrence runs in *exp space*:
    A_{t+1}[nxt, b] = exp(x_{t+1}[nxt, b] - d) * sum_prev E[prev, nxt] * A_t[prev, b]
with E = exp(transition) as the PE stationary operand and a constant
per-step normalizer d (mean log-growth rate) keeping A in f32 range.  Each
step is exactly one matmul (PE, float32r at full rate) + one elementwise
multiply (DVE).

The serial chain over T is broken by chunked speculation: T splits into
C=32 chunks of S=16 steps; each chunk warms up for K=6 steps on the
preceding chunk's data from a uniform state (the forward state forgets its
init to <1e-7 in ~10 steps), after which its state equals the true state up
to a per-batch scalar.  The scalars are recovered exactly from per-chunk
column sums (ones-matmuls) and telescoped on the host in f64.  All 32
chunks advance in lockstep (22 slots instead of 512 steps), stacked as two
[128, 8*32] supergroups; a block-diagonal [[E,0],[0,E]] stationary keeps
the two 64-row halves independent in one full-K matmul.

The masked transition column (into 'B', exactly -10000 => exp == 0 in f32)
makes alpha[:, 0] equal -inf in exp space; it is reconstructed exactly as
-10000 + lse(alpha_{T-1}) + x_{T-1, 0} from an extra ones-matmul.

walrus in this pipeline encodes at most ONE semaphore wait per regular
instruction (Bacc splits the rest, at a latency cost), so the program keeps
a strict 1-wait discipline: a barrier right after the bundle DMA, exp
pieces matched 1:1 with their DMA pieces, and same-engine observer ops
("dummies") so each DVE multiply joins only the PE stream.
"""

import numpy as np
from contextlib import ExitStack

import concourse.bacc as bacc
import concourse.tile as tile
from concourse import mybir
from concourse.bass_utils import run_bass_kernel_spmd
from concourse.tile_rust import add_dep_helper

F32 = mybir.dt.float32
F32R = mybir.dt.float32r
EXP = mybir.ActivationFunctionType.Exp

NCORES = 8
B, T, L = 256, 512, 64
BC = B // NCORES          # batch per core = 32
C = 32                    # chunks
S = T // C                # steps per chunk = 16
K = 6                     # warm-up steps
SL = K + S                # lockstep slots = 22
SG = 2                    # supergroups: 16 chunks each (8 per 64-row half)
SGW = 8                   # chunks per supergroup row-half
NEG = -10000.0

# X device layout, flat [128, 8448] per core:
#   region w0 [0, 2176):    kk-major upper-half stripes k8 in [0,4):
#                           (kk, block j in [0,17), b)
#   region w1 [2176, 4352): upper-half stripes k8 in [4,8)
#   region w2 [4352, 6400): block-major lower-half: (block 1..16, m in [0,4), b)
#   region w3 [6400, 8448): lower-half stripes m in [4,8)
# "block j upper stripe k8" holds t_loc = 16(j-1)+8+k8 (A rows; zeros for
# t_loc < 0) and 256+t_loc (B rows); "block j+1 lower stripe m" holds
# t_loc = 16j+m.  Slot k in [0, SL) corresponds to k8 = k + (8 - K).
W01 = 2176
W23 = 2048
NCOL = 2 * W01 + 2 * W23  # 8448
K8OFF = 8 - K


def _mult_src(k, s):
    """Source of the Ex slice for supergroup s at slot k.
    Returns ("w01", flat_col) for 2D slices of the w0/w1 regions, or
    ("w23", window, block0, u0) for 3D slices of the w2/w3 tiles."""
    k8 = k + K8OFF
    if k8 < 8:
        w, kk = k8 // 4, k8 % 4
        return ("w01", w, kk * 544 + (SGW * s) * 32)
    elif k8 < 16:
        w = (k8 - 8) // 4
        return ("w23", w, SGW * s, 32 * ((k8 - 8) % 4))
    else:
        w, kk = (k8 - 16) // 4, (k8 - 16) % 4
        return ("w01", w, kk * 544 + (SGW * s + 1) * 32)


def _build_program():
    nc = bacc.Bacc("TRN2", target_bir_lowering=False, debug=False,
                   num_devices=NCORES)
    xt_ap = nc.dram_tensor("xt", [128, NCOL], F32, kind="ExternalInput").ap()
    bd_ap = nc.dram_tensor("bundle", [128, 164], F32, kind="ExternalInput").ap()
    af_ap = nc.dram_tensor("afinal", [64, 1, 32], F32, kind="ExternalOutput").ap()
    sm_ap = nc.dram_tensor("sums", [2, 1056], F32, kind="ExternalOutput").ap()

    with tile.TileContext(nc) as tc, ExitStack() as ctx:
        pc = ctx.enter_context(tc.tile_pool(name="const", bufs=1))
        px = ctx.enter_context(tc.tile_pool(name="x", bufs=1))
        pst = [ctx.enter_context(tc.tile_pool(name=f"st{s}", bufs=3))
               for s in range(SG)]
        pps = [ctx.enter_context(tc.tile_pool(name=f"ps{s}", bufs=1, space="PSUM"))
               for s in range(SG)]
        psm = ctx.enter_context(tc.tile_pool(name="psums", bufs=1, space="PSUM"))

        # ---- preamble (no barrier: strict 1-wait discipline throughout) ----
        # prewarm the ACT function table so the 1.3us LoadActFuncSet overlaps
        # the input DMA instead of delaying the first real exp
        actwarm = pc.tile([1, 4], F32)
        nc.scalar.activation(actwarm[0:1, 0:1],
                             nc.const_aps.tensor(1.0, (1, 1)), EXP,
                             bias=0.0, scale=1.0)

        bundle = pc.tile([128, 164], F32)
        xw01 = [px.tile([128, W01], F32, name=f"xw01_{i}", tag=f"xw01_{i}") for i in range(2)]
        # first warm-up stripe ahead of the bundle on the SP ring: slot 0
        # needs both, and the stripe is the longer pole
        w0_, kk0_ = 0, 2
        nc.sync.dma_start(xw01[w0_][:, kk0_ * 544:(kk0_ + 1) * 544],
                          xt_ap[:, w0_ * W01 + kk0_ * 544:w0_ * W01 + (kk0_ + 1) * 544])
        nc.sync.dma_start(bundle[:], bd_ap)
        # bundle cols: 0:128 block-diagonal transition (off-diag -1e4 so exp
        # gives exact zeros), 128:160 onehot reset, 160:162 half-ones pair,
        # 162 = -d bias

        states = []
        for s in range(SG):
            t0 = pst[s].tile([128, SGW, 32], F32, tag="st")
            nc.vector.memset(t0[:], 1.0)
            states.append(t0)
        collect = pc.tile([2, 1056], F32)
        scratch = pc.tile([1, 4], F32)
        nc.vector.memset(scratch[:], 0.0)

        # E = exp(block-diag transition); f32r-rounded for the f32r matmuls
        E = pc.tile([128, 128], F32)
        nc.scalar.activation(E[:].bitcast(F32R), bundle[:, 0:128], EXP,
                             bias=0.0, scale=1.0)

        # ---- X streaming: one SP ring, strict consumption order ----
        ew01 = [px.tile([128, W01], F32, name=f"ew01_{i}", tag=f"ew01_{i}") for i in range(2)]
        xw23 = [px.tile([128, 16, 128], F32, name=f"xw23_{i}", tag=f"xw23_{i}") for i in range(2)]
        ew23 = [px.tile([128, 16, 128], F32, name=f"ew23_{i}", tag=f"ew23_{i}") for i in range(2)]

        bias_ap = bundle[:, 162:163]
        early = [(0, 2), (0, 3), (1, 0), (1, 1), (1, 2), (1, 3)]
        late = [(0, 0), (0, 1)]

        def dma_stripe(w, kk):
            nc.sync.dma_start(xw01[w][:, kk * 544:(kk + 1) * 544],
                              xt_ap[:, w * W01 + kk * 544:w * W01 + (kk + 1) * 544])

        def exp_stripe(w, kk):
            nc.scalar.activation(
                ew01[w][:, kk * 544:(kk + 1) * 544],
                xw01[w][:, kk * 544:(kk + 1) * 544], EXP,
                bias=bias_ap, scale=1.0)

        def dma_w23(w, g):
            nc.sync.dma_start(
                xw23[w][:, 8 * g:8 * g + 8, :],
                xt_ap[:, 2 * W01 + w * W23 + g * 1024:
                       2 * W01 + w * W23 + (g + 1) * 1024]
                .rearrange("p (j u) -> p j u", u=128))

        def exp_w23(w, g):
            nc.scalar.activation(
                ew23[w][:, 8 * g:8 * g + 8, :],
                xw23[w][:, 8 * g:8 * g + 8, :], EXP,
                bias=bias_ap, scale=1.0)

        for w, kk in early[1:]:
            dma_stripe(w, kk)
        for w in range(2):
            for g in range(SG):
                dma_w23(w, g)
        for w, kk in late:
            dma_stripe(w, kk)

        for w, kk in early:
            exp_stripe(w, kk)
        for w in range(2):
            for g in range(SG):
                exp_w23(w, g)
        for w, kk in late:
            exp_stripe(w, kk)

        # last-emitted exp piece observed by each slot's dummy (ACT ticks are
        # monotone in emission order, so observing a later piece covers all
        # earlier ones)
        dummy_reads = {}
        for i in range(K):
            w, kk = early[i]
            dummy_reads[i] = ew01[w][0:1, kk * 544 + 543:kk * 544 + 544]
        dummy_reads[K] = ew23[0][0:1, 15, 127:128]
        dummy_reads[K + 4] = ew23[1][0:1, 15, 127:128]
        dummy_reads[K + 8] = ew01[0][0:1, 1 * 544 + 543:1 * 544 + 544]

        # ---- main lockstep loop ----
        ones2 = bundle[:, 160:162]
        start_ps = psm.tile([2, 512], F32)
        end_ps = psm.tile([2, 512], F32)
        preT_ps = psm.tile([2, 32], F32)

        # junk matmuls: PE observes the bundle DMA tick, then the DVE memset
        # tick, so later PE instructions each carry at most one wait
        nc.tensor.matmul(start_ps[0:1, 0:164], lhsT=bundle[0:64, 160:161],
                         rhs=bundle[0:64, :], start=True, stop=True)
        nc.tensor.matmul(start_ps[0:1, 0:256], lhsT=states[0][0:64, 0, 0:1],
                         rhs=states[0][0:64], start=True, stop=True)

        prevT1 = None
        for k in range(SL):
            dummy = None
            if k in dummy_reads:
                dummy = nc.vector.tensor_copy(scratch[0:1, 0:1], dummy_reads[k])
            new_states = []
            for s in range(SG):
                ps = pps[s].tile([128, SGW, 32], F32, tag="ps")
                nc.tensor.matmul(ps[:], lhsT=E[:].bitcast(F32R),
                                 rhs=states[s][:].bitcast(F32R),
                                 start=True, stop=True)
                src = _mult_src(k, s)
                if src[0] == "w01":
                    _, w, col = src
                    in1 = ew01[w][:, col:col + 256].rearrange(
                        "p (j m) -> p j m", m=32)
                else:
                    _, w, b0, u0 = src
                    in1 = ew23[w][:, b0:b0 + SGW, u0:u0 + 32]
                nst = pst[s].tile([128, SGW, 32], F32, tag="st")
                m = nc.vector.tensor_mul(nst[:].bitcast(F32R), ps[:], in1)
                if dummy is not None:
                    add_dep_helper(m.ins, dummy.ins, sync=False,
                                   reason="observe new exp piece first")
                new_states.append(nst)
            states = new_states

            if k == K - 1:
                # reset chunk 0 to the exact one-hot init, then record
                # per-chunk start sums
                nc.vector.tensor_copy(states[0][0:64, 0, :].bitcast(F32R),
                                      bundle[0:64, 128:160])
                for s in range(SG):
                    nc.tensor.matmul(start_ps[0:2, s * 256:(s + 1) * 256],
                                     lhsT=ones2, rhs=states[s][:],
                                     start=True, stop=True)
                nc.vector.tensor_copy(collect[0:2, 0:512], start_ps[0:2, :])
            if k == SL - 2:
                prevT1 = states[SG - 1]

        # preT: chunk 31 state before its last step
        nc.tensor.matmul(preT_ps[0:2, :], lhsT=ones2, rhs=prevT1[:, 7, :],
                         start=True, stop=True)
        for s in range(SG):
            nc.tensor.matmul(end_ps[0:2, s * 256:(s + 1) * 256],
                             lhsT=ones2, rhs=states[s][:],
                             start=True, stop=True)
        nc.vector.tensor_copy(collect[0:2, 512:1024], end_ps[0:2, :])
        nc.vector.tensor_copy(collect[0:2, 1024:1056], preT_ps[0:2, :])

        nc.sync.dma_start(af_ap, states[SG - 1][64:128, 7:8, :])
        nc.sync.dma_start(sm_ap, collect[:])
    nc.compile()
    return nc


_prog_cache = {}


def _get_program():
    if "nc" not in _prog_cache:
        _prog_cache["nc"] = _build_program()
    return _prog_cache["nc"]


def _compute_d(X, transition):
    """Mean per-step log growth of the total exp-space mass, from a short
    host-side probe.  Any value within ~+-0.1 keeps A in f32 range."""
    E = np.exp(transition.astype(np.float64))
    a = np.zeros((16, L), np.float64)
    a[:, 0] = 1.0
    tot, n = 0.0, 0
    for t in range(96):
        a = np.exp(X[:16, t, :].astype(np.float64)) * (a @ E)
        sm = a.sum()
        a /= sm
        if t >= 4:
            tot += np.log(sm)
            n += 1
    return float(np.clip(tot / n, 4.5, 5.9))


def _pack_core(Xc, d):
    """Xc [32, T, L] -> device layout [128, NCOL] f32 (see module header)."""
    Y = np.ascontiguousarray(Xc.transpose(2, 1, 0)).astype(np.float32)  # [L, T, 32]
    out = np.zeros((128, NCOL), np.float32)
    # windows 0,1: kk-major upper-half stripes
    for w in (0, 1):
        for kk in range(4):
            k8 = 4 * w + kk
            for j in range(17):
                t_loc = 16 * (j - 1) + 8 + k8
                dst = out[:, w * W01 + kk * 544 + j * 32:
                          w * W01 + kk * 544 + (j + 1) * 32]
                if t_loc >= 0:
                    dst[0:64] = Y[:, t_loc, :]
                dst[64:128] = Y[:, 256 + t_loc, :]
    # windows 2,3: block-major lower-half stripes
    for w in (2, 3):
        for j in range(16):
            for kk in range(4):
                m = 4 * (w - 2) + kk
                t_loc = 16 * j + m
                base = 2 * W01 + (w - 2) * W23 + j * 128 + kk * 32
                dst = out[:, base:base + 32]
                dst[0:64] = Y[:, t_loc, :]
                dst[64:128] = Y[:, 256 + t_loc, :]
    return out


def _make_bundle(transition, d):
    bd = np.zeros((128, 164), np.float32)
    tr = transition.astype(np.float32)
    bd[:, 0:128] = NEG            # off-diagonal blocks -> exp == 0 exactly
    bd[0:64, 0:64] = tr
    bd[64:128, 64:128] = tr
    bd[0, 128:160] = 1.0          # one-hot reset block: row B_IDX=0
    bd[0:64, 160] = 1.0           # half-ones pair for partition sums
    bd[64:128, 161] = 1.0
    bd[:, 162] = -d               # exp bias
    return bd


def kernel(X, transition):
    X = np.asarray(X, dtype=np.float32)
    transition = np.asarray(transition, dtype=np.float32)
    d = _compute_d(X, transition)

    bundle = _make_bundle(transition, d)
    in_maps = []
    for c in range(NCORES):
        xt = _pack_core(X[c * BC:(c + 1) * BC], d)
        in_maps.append({"xt": xt, "bundle": bundle})

    nc = _get_program()
    res = run_bass_kernel_spmd(nc, in_maps, core_ids=list(range(NCORES)))

    alpha = np.empty((B, L), np.float64)
    dS = float(d) * S
    with np.errstate(divide="ignore"):
        for c in range(NCORES):
            r = res.results[c]
            sums = r["sums"].astype(np.float64)
            af = r["afinal"].reshape(64, 32).astype(np.float64)
            start = np.empty((C, BC))
            end = np.empty((C, BC))
            start[:16] = sums[0, 0:512].reshape(16, BC)
            start[16:] = sums[1, 0:512].reshape(16, BC)
            end[:16] = sums[0, 512:1024].reshape(16, BC)
            end[16:] = sums[1, 512:1024].reshape(16, BC)
            preT = sums[1, 1024:1056]
            lam = np.zeros(BC)
            for cc in range(C - 1):
                lam += dS + np.log(end[cc]) - np.log(start[cc])
            base = lam - np.log(start[C - 1])
            blk = alpha[c * BC:(c + 1) * BC]
            blk[:] = (base[:, None] + dS + np.log(af).T)
            lse_preT = base + (dS - d) + np.log(preT)
            blk[:, 0] = NEG + lse_preT + X[c * BC:(c + 1) * BC, T - 1, 0].astype(np.float64)
    return alpha.astype(np.float32)


# revision 19
# speedup vs baseline: 1.0541x; 1.0541x over previous
"""CRF forward (alpha) recursion on 8 Trainium2 NeuronCores.

Strategy
--------
Data-parallel over batch: each core gets 32 of 256 batch rows.

Per core, the T=512 recurrence runs in *exp space*:
    A_{t+1}[nxt, b] = exp(x_{t+1}[nxt, b] - d) * sum_prev E[prev, nxt] * A_t[prev, b]
with E = exp(transition) as the PE stationary operand and a constant
per-step normalizer d (mean log-growth rate) keeping A in f32 range.  Each
step is exactly one matmul (PE, float32r at full rate) + one elementwise
multiply (DVE).

The serial chain over T is broken by chunked speculation: T splits into
C=32 chunks of S=16 steps; each chunk warms up for K=6 steps on the
preceding chunk's data from a uniform state (the forward state forgets its
init to <1e-7 in ~10 steps), after which its state equals the true state up
to a per-batch scalar.  The scalars are recovered exactly from per-chunk
column sums (ones-matmuls) and telescoped on the host in f64.  All 32
chunks advance in lockstep (22 slots instead of 512 steps), stacked as two
[128, 8*32] supergroups; a block-diagonal [[E,0],[0,E]] stationary keeps
the two 64-row halves independent in one full-K matmul.

The masked transition column (into 'B', exactly -10000 => exp == 0 in f32)
makes alpha[:, 0] equal -inf in exp space; it is reconstructed exactly as
-10000 + lse(alpha_{T-1}) + x_{T-1, 0} from an extra ones-matmul.

walrus in this pipeline encodes at most ONE semaphore wait per regular
instruction (Bacc splits the rest, at a latency cost), so the program keeps
a strict 1-wait discipline: a barrier right after the bundle DMA, exp
pieces matched 1:1 with their DMA pieces, and same-engine observer ops
("dummies") so each DVE multiply joins only the PE stream.
"""

import numpy as np
from contextlib import ExitStack

import concourse.bacc as bacc
import concourse.tile as tile
from concourse import mybir
from concourse.bass_utils import run_bass_kernel_spmd
from concourse.tile_rust import add_dep_helper

F32 = mybir.dt.float32
F32R = mybir.dt.float32r
EXP = mybir.ActivationFunctionType.Exp

NCORES = 8
B, T, L = 256, 512, 64
BC = B // NCORES          # batch per core = 32
C = 32                    # chunks
S = T // C                # steps per chunk = 16
K = 4                     # warm-up steps
SL = K + S                # lockstep slots = 22
SG = 2                    # supergroups: 16 chunks each (8 per 64-row half)
SGW = 8                   # chunks per supergroup row-half
NEG = -10000.0

# X device layout, flat [128, 8448] per core:
#   region w0 [0, 2176):    kk-major upper-half stripes k8 in [0,4):
#                           (kk, block j in [0,17), b)
#   region w1 [2176, 4352): upper-half stripes k8 in [4,8)
#   region w2 [4352, 6400): block-major lower-half: (block 1..16, m in [0,4), b)
#   region w3 [6400, 8448): lower-half stripes m in [4,8)
# "block j upper stripe k8" holds t_loc = 16(j-1)+8+k8 (A rows; zeros for
# t_loc < 0) and 256+t_loc (B rows); "block j+1 lower stripe m" holds
# t_loc = 16j+m.  Slot k in [0, SL) corresponds to k8 = k + (8 - K).
W01 = 2176
W23 = 2048
NCOL = 2 * W01 + 2 * W23  # 8448
K8OFF = 8 - K


def _mult_src(k, s):
    """Source of the Ex slice for supergroup s at slot k.
    Returns ("w01", flat_col) for 2D slices of the w0/w1 regions, or
    ("w23", window, block0, u0) for 3D slices of the w2/w3 tiles."""
    k8 = k + K8OFF
    if k8 < 8:
        w, kk = k8 // 4, k8 % 4
        return ("w01", w, kk * 544 + (SGW * s) * 32)
    elif k8 < 16:
        w = (k8 - 8) // 4
        return ("w23", w, SGW * s, 32 * ((k8 - 8) % 4))
    else:
        w, kk = (k8 - 16) // 4, (k8 - 16) % 4
        return ("w01", w, kk * 544 + (SGW * s + 1) * 32)


def _build_program():
    nc = bacc.Bacc("TRN2", target_bir_lowering=False, debug=False,
                   num_devices=NCORES)
    xt_ap = nc.dram_tensor("xt", [128, NCOL], F32, kind="ExternalInput").ap()
    bd_ap = nc.dram_tensor("bundle", [128, 164], F32, kind="ExternalInput").ap()
    af_ap = nc.dram_tensor("afinal", [64, 1, 32], F32, kind="ExternalOutput").ap()
    sm_ap = nc.dram_tensor("sums", [2, 1056], F32, kind="ExternalOutput").ap()

    with tile.TileContext(nc) as tc, ExitStack() as ctx:
        pc = ctx.enter_context(tc.tile_pool(name="const", bufs=1))
        px = ctx.enter_context(tc.tile_pool(name="x", bufs=1))
        pst = [ctx.enter_context(tc.tile_pool(name=f"st{s}", bufs=3))
               for s in range(SG)]
        pps = [ctx.enter_context(tc.tile_pool(name=f"ps{s}", bufs=1, space="PSUM"))
               for s in range(SG)]
        psm = ctx.enter_context(tc.tile_pool(name="psums", bufs=1, space="PSUM"))

        # ---- preamble (no barrier: strict 1-wait discipline throughout) ----
        # prewarm the ACT function table so the 1.3us LoadActFuncSet overlaps
        # the input DMA instead of delaying the first real exp
        actwarm = pc.tile([1, 4], F32)
        nc.scalar.activation(actwarm[0:1, 0:1],
                             nc.const_aps.tensor(1.0, (1, 1)), EXP,
                             bias=0.0, scale=1.0)

        bundle = pc.tile([128, 164], F32)
        xw01 = [px.tile([128, W01], F32, name=f"xw01_{i}", tag=f"xw01_{i}") for i in range(2)]
        # first warm-up stripe ahead of the bundle on the SP ring: slot 0
        # needs both, and the stripe is the longer pole
        w0_, kk0_ = (K8OFF // 4, K8OFF % 4)
        nc.sync.dma_start(xw01[w0_][:, kk0_ * 544:(kk0_ + 1) * 544],
                          xt_ap[:, w0_ * W01 + kk0_ * 544:w0_ * W01 + (kk0_ + 1) * 544])
        nc.sync.dma_start(bundle[:], bd_ap)
        # bundle cols: 0:128 block-diagonal transition (off-diag -1e4 so exp
        # gives exact zeros), 128:160 onehot reset, 160:162 half-ones pair,
        # 162 = -d bias

        states = []
        for s in range(SG):
            t0 = pst[s].tile([128, SGW, 32], F32, tag="st")
            nc.vector.memset(t0[:], 1.0)
            states.append(t0)
        collect = pc.tile([2, 1056], F32)
        scratch = pc.tile([1, 4], F32)
        nc.vector.memset(scratch[:], 0.0)

        # E = exp(block-diag transition); f32r-rounded for the f32r matmuls
        E = pc.tile([128, 128], F32)
        nc.scalar.activation(E[:].bitcast(F32R), bundle[:, 0:128], EXP,
                             bias=0.0, scale=1.0)

        # ---- X streaming: one SP ring, strict consumption order ----
        ew01 = [px.tile([128, W01], F32, name=f"ew01_{i}", tag=f"ew01_{i}") for i in range(2)]
        xw23 = [px.tile([128, 16, 128], F32, name=f"xw23_{i}", tag=f"xw23_{i}") for i in range(2)]
        ew23 = [px.tile([128, 16, 128], F32, name=f"ew23_{i}", tag=f"ew23_{i}") for i in range(2)]

        bias_ap = bundle[:, 162:163]
        # upper-half stripes consumed during warm-up (k8 in [K8OFF, 8)) come
        # first; the rest are only re-read from slot k8=16 onward
        early = [(k8 // 4, k8 % 4) for k8 in range(K8OFF, 8)]
        late = [(k8 // 4, k8 % 4) for k8 in range(K8OFF)]

        def dma_stripe(w, kk):
            nc.sync.dma_start(xw01[w][:, kk * 544:(kk + 1) * 544],
                              xt_ap[:, w * W01 + kk * 544:w * W01 + (kk + 1) * 544])

        def exp_stripe(w, kk):
            nc.scalar.activation(
                ew01[w][:, kk * 544:(kk + 1) * 544],
                xw01[w][:, kk * 544:(kk + 1) * 544], EXP,
                bias=bias_ap, scale=1.0)

        def dma_w23(w, g):
            nc.sync.dma_start(
                xw23[w][:, 8 * g:8 * g + 8, :],
                xt_ap[:, 2 * W01 + w * W23 + g * 1024:
                       2 * W01 + w * W23 + (g + 1) * 1024]
                .rearrange("p (j u) -> p j u", u=128))

        def exp_w23(w, g):
            nc.scalar.activation(
                ew23[w][:, 8 * g:8 * g + 8, :],
                xw23[w][:, 8 * g:8 * g + 8, :], EXP,
                bias=bias_ap, scale=1.0)

        for w, kk in early[1:]:
            dma_stripe(w, kk)
        for w in range(2):
            for g in range(SG):
                dma_w23(w, g)
        for w, kk in late:
            dma_stripe(w, kk)

        for w, kk in early:
            exp_stripe(w, kk)
        for w in range(2):
            for g in range(SG):
                exp_w23(w, g)
        for w, kk in late:
            exp_stripe(w, kk)

        # last-emitted exp piece observed by each slot's dummy (ACT ticks are
        # monotone in emission order, so observing a later piece covers all
        # earlier ones)
        dummy_reads = {}
        for i in range(K):
            w, kk = early[i]
            dummy_reads[i] = ew01[w][0:1, kk * 544 + 543:kk * 544 + 544]
        dummy_reads[K] = ew23[0][0:1, 15, 127:128]
        dummy_reads[K + 4] = ew23[1][0:1, 15, 127:128]
        lw, lkk = late[-1]
        dummy_reads[K + 8] = ew01[lw][0:1, lkk * 544 + 543:lkk * 544 + 544]

        # ---- main lockstep loop ----
        ones2 = bundle[:, 160:162]
        start_ps = psm.tile([2, 512], F32)
        end_ps = psm.tile([2, 512], F32)
        preT_ps = psm.tile([2, 32], F32)

        # junk matmuls: PE observes the bundle DMA tick, then the DVE memset
        # tick, so later PE instructions each carry at most one wait
        nc.tensor.matmul(start_ps[0:1, 0:164], lhsT=bundle[0:64, 160:161],
                         rhs=bundle[0:64, :], start=True, stop=True)
        nc.tensor.matmul(start_ps[0:1, 0:256], lhsT=states[0][0:64, 0, 0:1],
                         rhs=states[0][0:64], start=True, stop=True)

        prevT1 = None
        for k in range(SL):
            dummy = None
            if k in dummy_reads:
                dummy = nc.vector.tensor_copy(scratch[0:1, 0:1], dummy_reads[k])
            new_states = []
            for s in range(SG):
                ps = pps[s].tile([128, SGW, 32], F32, tag="ps")
                nc.tensor.matmul(ps[:], lhsT=E[:].bitcast(F32R),
                                 rhs=states[s][:].bitcast(F32R),
                                 start=True, stop=True)
                src = _mult_src(k, s)
                if src[0] == "w01":
                    _, w, col = src
                    in1 = ew01[w][:, col:col + 256].rearrange(
                        "p (j m) -> p j m", m=32)
                else:
                    _, w, b0, u0 = src
                    in1 = ew23[w][:, b0:b0 + SGW, u0:u0 + 32]
                nst = pst[s].tile([128, SGW, 32], F32, tag="st")
                m = nc.vector.tensor_mul(nst[:].bitcast(F32R), ps[:], in1)
                if dummy is not None:
                    add_dep_helper(m.ins, dummy.ins, sync=False,
                                   reason="observe new exp piece first")
                new_states.append(nst)
            states = new_states

            if k == K - 1:
                # reset chunk 0 to the exact one-hot init, then record
                # per-chunk start sums
                nc.vector.tensor_copy(states[0][0:64, 0, :].bitcast(F32R),
                                      bundle[0:64, 128:160])
                for s in range(SG):
                    nc.tensor.matmul(start_ps[0:2, s * 256:(s + 1) * 256],
                                     lhsT=ones2, rhs=states[s][:],
                                     start=True, stop=True)
                nc.vector.tensor_copy(collect[0:2, 0:512], start_ps[0:2, :])
            if k == SL - 2:
                prevT1 = states[SG - 1]

        # preT: chunk 31 state before its last step
        nc.tensor.matmul(preT_ps[0:2, :], lhsT=ones2, rhs=prevT1[:, 7, :],
                         start=True, stop=True)
        for s in range(SG):
            nc.tensor.matmul(end_ps[0:2, s * 256:(s + 1) * 256],
                             lhsT=ones2, rhs=states[s][:],
                             start=True, stop=True)
        nc.vector.tensor_copy(collect[0:2, 512:1024], end_ps[0:2, :])
        nc.vector.tensor_copy(collect[0:2, 1024:1056], preT_ps[0:2, :])

        nc.sync.dma_start(af_ap, states[SG - 1][64:128, 7:8, :])
        nc.sync.dma_start(sm_ap, collect[:])
    nc.compile()
    return nc


_prog_cache = {}


def _get_program():
    if "nc" not in _prog_cache:
        _prog_cache["nc"] = _build_program()
    return _prog_cache["nc"]


def _compute_d(X, transition):
    """Mean per-step log growth of the total exp-space mass, from a short
    host-side probe.  Any value within ~+-0.1 keeps A in f32 range."""
    E = np.exp(transition.astype(np.float64))
    a = np.zeros((16, L), np.float64)
    a[:, 0] = 1.0
    tot, n = 0.0, 0
    for t in range(96):
        a = np.exp(X[:16, t, :].astype(np.float64)) * (a @ E)
        sm = a.sum()
        a /= sm
        if t >= 4:
            tot += np.log(sm)
            n += 1
    return float(np.clip(tot / n, 4.5, 5.9))


def _pack_core(Xc, d):
    """Xc [32, T, L] -> device layout [128, NCOL] f32 (see module header)."""
    Y = np.ascontiguousarray(Xc.transpose(2, 1, 0)).astype(np.float32)  # [L, T, 32]
    out = np.zeros((128, NCOL), np.float32)
    # windows 0,1: kk-major upper-half stripes
    for w in (0, 1):
        for kk in range(4):
            k8 = 4 * w + kk
            for j in range(17):
                t_loc = 16 * (j - 1) + 8 + k8
                dst = out[:, w * W01 + kk * 544 + j * 32:
                          w * W01 + kk * 544 + (j + 1) * 32]
                if t_loc >= 0:
                    dst[0:64] = Y[:, t_loc, :]
                dst[64:128] = Y[:, 256 + t_loc, :]
    # windows 2,3: block-major lower-half stripes
    for w in (2, 3):
        for j in range(16):
            for kk in range(4):
                m = 4 * (w - 2) + kk
                t_loc = 16 * j + m
                base = 2 * W01 + (w - 2) * W23 + j * 128 + kk * 32
                dst = out[:, base:base + 32]
                dst[0:64] = Y[:, t_loc, :]
                dst[64:128] = Y[:, 256 + t_loc, :]
    return out


def _make_bundle(transition, d):
    bd = np.zeros((128, 164), np.float32)
    tr = transition.astype(np.float32)
    bd[:, 0:128] = NEG            # off-diagonal blocks -> exp == 0 exactly
    bd[0:64, 0:64] = tr
    bd[64:128, 64:128] = tr
    bd[0, 128:160] = 1.0          # one-hot reset block: row B_IDX=0
    bd[0:64, 160] = 1.0           # half-ones pair for partition sums
    bd[64:128, 161] = 1.0
    bd[:, 162] = -d               # exp bias
    return bd


def kernel(X, transition):
    X = np.asarray(X, dtype=np.float32)
    transition = np.asarray(transition, dtype=np.float32)
    d = _compute_d(X, transition)

    bundle = _make_bundle(transition, d)
    in_maps = []
    for c in range(NCORES):
        xt = _pack_core(X[c * BC:(c + 1) * BC], d)
        in_maps.append({"xt": xt, "bundle": bundle})

    nc = _get_program()
    res = run_bass_kernel_spmd(nc, in_maps, core_ids=list(range(NCORES)))

    alpha = np.empty((B, L), np.float64)
    dS = float(d) * S
    with np.errstate(divide="ignore"):
        for c in range(NCORES):
            r = res.results[c]
            sums = r["sums"].astype(np.float64)
            af = r["afinal"].reshape(64, 32).astype(np.float64)
            start = np.empty((C, BC))
            end = np.empty((C, BC))
            start[:16] = sums[0, 0:512].reshape(16, BC)
            start[16:] = sums[1, 0:512].reshape(16, BC)
            end[:16] = sums[0, 512:1024].reshape(16, BC)
            end[16:] = sums[1, 512:1024].reshape(16, BC)
            preT = sums[1, 1024:1056]
            lam = np.zeros(BC)
            for cc in range(C - 1):
                lam += dS + np.log(end[cc]) - np.log(start[cc])
            base = lam - np.log(start[C - 1])
            blk = alpha[c * BC:(c + 1) * BC]
            blk[:] = (base[:, None] + dS + np.log(af).T)
            lse_preT = base + (dS - d) + np.log(preT)
            blk[:, 0] = NEG + lse_preT + X[c * BC:(c + 1) * BC, T - 1, 0].astype(np.float64)
    return alpha.astype(np.float32)
